# revision 28
# baseline (speedup 1.0000x reference)
"""LQLinear (2-bit learned VQ linear) Trainium2 kernel.

Math (Q_T=1): the least-squares basis refit only feeds the *discarded*
buffer update, so the forward output is

    out = x @ wq.T + bias

where wq bucketizes weight into the 4 sorted levels {+-b_small +- b_big}
(b_small, b_big = sorted |basis|), thresholds at midpoints {-b_big, 0, +b_big}.

Device strategy (8 cores, out_features-sharded, 512 rows each):
  - wq = b_small * wqn with wqn in {+-1, +-(b_big/b_small)}; for the
    reference basis (b_big = 2*b_small) wqn in {+-1, +-3}: EXACT in fp8/bf16.
  - greedy sign quantization == bucketize (decided in f32: bf16/f32r
    thresholds would flip ~0.4% of weights near +-b_big): s_big = sign(w),
    ss2 = sign(|w| - b_big), wqn = s_big * (R + ss2).
  - GEMM: stationary wqn (fp8e4 -> fast weight load), moving x in bf16
    (output rel err ~1.5e-3 from bf16 rounding of x), fp32 PSUM.
  - Quantize is pipelined per k-tile so the first MM group starts as soon
    as wq[kt=0] lands instead of after the full weight shard.
  - Host prep is layout-only sharding work (transpose/cast/block), so
    every device DMA is contiguous per partition; DMAs are split across
    both HW-DGE rings (sync + scalar).
  - DVE evicts PSUM with fused out = b_small*psum + bias[o].
"""

import os
import sys

for _p in ("/opt/trn_rl_repo", "/root/.axon_site/_ro/trn_rl_repo"):
    if os.path.isdir(_p) and _p not in sys.path:
        sys.path.insert(0, _p)

import numpy as np
import ml_dtypes

N_CORES = 8
TOKENS = 8192
IN_F = 4096
OUT_F = 4096
O_SHARD = OUT_F // N_CORES          # 512 output rows per core
KT = IN_F // 128                    # 32 k-tiles
KH = KT // 2                        # x streamed in half-k chunks of 16
TB = 512                            # token block (psum free dim)
N_TB = TOKENS // TB                 # 16 token blocks
O_SUB = O_SHARD // 128              # 4 output subtiles per core

LAST_RUN_INFO = {}


def _build_nc(b_small: float, b_big: float, wdt_name: str):
    import concourse.mybir as mybir
    import concourse.tile as tile
    from concourse import bacc

    dt = mybir.dt
    Alu = mybir.AluOpType

    R = b_big / b_small
    wdt = getattr(dt, wdt_name)     # lhsT dtype: float8e4 (default) or bfloat16

    nc = bacc.Bacc("TRN2", target_bir_lowering=False,
                   debug=os.environ.get("LQ_DEBUG", "0") == "1")

    # blocked, fully-contiguous-per-partition host layouts
    wT = nc.dram_tensor("wT", [KT, 128, O_SHARD], dt.float32, kind="ExternalInput")
    xh = nc.dram_tensor("xh", [N_TB, 2, 128, KH, TB], dt.bfloat16,
                        kind="ExternalInput")
    bs = nc.dram_tensor("bs", [128, O_SUB], dt.float32, kind="ExternalInput")
    oT = nc.dram_tensor("oT", [N_TB, O_SUB, 128, TB], dt.float32,
                        kind="ExternalOutput")

    wT_r = wT.ap()                  # [kt][128, 512]
    xh_r = xh.ap()                  # [tb][h][128, KH, 512]
    oT_r = oT.ap()                  # [tb][osb][128, 512]

    with tile.TileContext(nc) as tc:
        with (
            tc.tile_pool(name="const", bufs=1) as const,
            tc.tile_pool(name="wq", bufs=1) as wqp,
            tc.tile_pool(name="wload", bufs=8) as wload,
            tc.tile_pool(name="quant", bufs=3) as qp,
            tc.tile_pool(name="xhp", bufs=4) as xhp,
            tc.tile_pool(name="outp", bufs=8) as outp,
            tc.tile_pool(name="psum", bufs=8, space="PSUM") as psp,
        ):
            bias_sb = const.tile([128, O_SUB], dt.float32)
            nc.sync.dma_start(bias_sb[:], bs.ap())
            nbb2 = const.tile([128, 1], dt.float32, tag="nbb2")
            nc.vector.memset(nbb2[:], -float(np.float32(b_big) * np.float32(b_big)))

            # x prefetch for the first token blocks starts immediately,
            # racing the quantize pipeline below
            x_tiles = {}

            def fetch_x(tb):
                for h in range(2):
                    x_t = xhp.tile([128, KH, TB], dt.bfloat16, tag=f"xh{h}")
                    nc.sync.dma_start(x_t[:], xh_r[tb, h])
                    x_tiles[(tb, h)] = x_t

            # ---- quantize weight shard -> wqn {+-1,+-R}, one tile per kt
            # w-loads share the sync ring with x, interleaved ahead of the
            # x tb-fetches in groups of 8 so the 2KB w packets are not
            # starved by the 16KB x packets (SDMA round-robins per packet).
            wq_t = []

            def quantize_w(kt):
                w_t = wload.tile([128, O_SHARD], dt.float32, tag="wl")
                nc.sync.dma_start(w_t[:], wT_r[kt])
                sb = qp.tile([128, O_SHARD], dt.float32, tag="sb")
                av = qp.tile([128, O_SHARD], dt.float32, tag="av")
                # ss2 = sign(|w| - b_big) computed as sign(w^2 - b_big^2)
                # (w^2 on DVE so ACT only does 2 ops per k-tile)
                nc.vector.tensor_tensor(av[:], w_t[:], w_t[:], Alu.mult)
                nc.scalar.sign(sb[:], w_t[:])
                nc.scalar.sign(av[:], av[:], bias=nbb2[:])
                # DVE: wqn = s_big * (R + ss2)  in {+-(R-1), +-(R+1)}
                nc.vector.tensor_scalar(av[:], av[:], R, None, Alu.add)
                wq = wqp.tile([128, O_SHARD], wdt, tag=f"wq{kt}")
                nc.vector.tensor_tensor(wq[:], sb[:], av[:], Alu.mult)
                wq_t.append(wq)

            for kt in range(8):
                quantize_w(kt)
            fetch_x(0)
            for kt in range(8, 16):
                quantize_w(kt)
            fetch_x(1)
            for kt in range(16, 24):
                quantize_w(kt)
            fetch_x(2)
            for kt in range(24, KT):
                quantize_w(kt)

            def evict(tb, osb, ps):
                o_t = outp.tile([128, TB], dt.float32, tag="ot")
                # out = b_small * psum + bias  (per-partition bias AP)
                nc.vector.tensor_scalar(o_t[:], ps[:], float(b_small),
                                        bias_sb[:, osb:osb + 1],
                                        Alu.mult, Alu.add)
                nc.scalar.dma_start(oT_r[tb, osb], o_t[:])

            # ---- GEMM  psum[o128, t512] += wqn[k,o].T @ xT[k,t]
            # tb=0 runs kt-bursts of 8 across its 4 groups so the PE keeps
            # working as k-tiles emerge from the quantize pipeline instead
            # of FIFO-stalling behind one group's next wq tile.
            xts0 = (x_tiles.pop((0, 0)), x_tiles.pop((0, 1)))
            ps0 = [psp.tile([128, TB], dt.float32, tag="ps", name=f"ps0{osb}")
                   for osb in range(O_SUB)]
            for b in range(KT // 8):
                for osb in range(O_SUB):
                    for kt in range(8 * b, 8 * b + 8):
                        nc.tensor.matmul(
                            ps0[osb][:],
                            wq_t[kt][:, osb * 128:(osb + 1) * 128],
                            xts0[kt // KH][:, kt % KH, :],
                            start=(kt == 0), stop=(kt == KT - 1))
            for osb in range(O_SUB):
                evict(0, osb, ps0[osb])

            for tb in range(1, N_TB):
                if tb + 2 < N_TB:
                    fetch_x(tb + 2)
                xts = (x_tiles.pop((tb, 0)), x_tiles.pop((tb, 1)))
                for osb in range(O_SUB):
                    ps = psp.tile([128, TB], dt.float32, tag="ps", name="ps")
                    for kt in range(KT):
                        nc.tensor.matmul(
                            ps[:],
                            wq_t[kt][:, osb * 128:(osb + 1) * 128],
                            xts[kt // KH][:, kt % KH, :],
                            start=(kt == 0), stop=(kt == KT - 1))
                    evict(tb, osb, ps)

    nc.compile()
    return nc


def kernel(x, weight, bias, basis):
    from concourse import bass_utils

    x = np.asarray(x, dtype=np.float32)
    weight = np.asarray(weight, dtype=np.float32)
    bias = np.asarray(bias, dtype=np.float32)
    basis = np.asarray(basis, dtype=np.float32)

    b_small, b_big = sorted(float(v) for v in np.abs(basis))
    wdt_name = os.environ.get("LQ_WDT", "float8e4")

    # ---- host-side shard/layout prep (transpose, cast, block)
    # xb[tb, h, p, kt, t] = x[tb*512+t, (h*16+kt)*128+p]
    xb = np.ascontiguousarray(
        x.T.reshape(2, KH, 128, N_TB, TB).transpose(3, 0, 2, 1, 4)
    ).astype(ml_dtypes.bfloat16)
    wt = weight.T                                        # [4096 in, 4096 out]

    in_maps = []
    for c in range(N_CORES):
        wb = np.ascontiguousarray(
            wt[:, c * O_SHARD:(c + 1) * O_SHARD]).reshape(KT, 128, O_SHARD)
        m = {
            "wT": wb,
            "xh": xb,
            "bs": np.ascontiguousarray(
                bias[c * O_SHARD:(c + 1) * O_SHARD].reshape(O_SUB, 128).T),
        }
        in_maps.append(m)

    nc = _build_nc(b_small, b_big, wdt_name)
    trace = os.environ.get("LQ_TRACE", "") == "1"
    res = bass_utils.run_bass_kernel_spmd(
        nc, in_maps, core_ids=list(range(N_CORES)), trace=trace)

    LAST_RUN_INFO.clear()
    LAST_RUN_INFO["exec_time_ns"] = res.exec_time_ns
    LAST_RUN_INFO["profile_json"] = res.profile_json
    LAST_RUN_INFO["nc"] = nc
    LAST_RUN_INFO["in_maps"] = in_maps

    # oT blocked [tb, osb, p, t] -> rows osb*128+p of the shard, cols tb*512+t
    outT = np.concatenate(
        [res.results[c]["oT"].transpose(1, 2, 0, 3).reshape(O_SHARD, TOKENS)
         for c in range(N_CORES)], axis=0)
    return np.ascontiguousarray(outT.T).astype(np.float32)
